# revision 14
# baseline (speedup 1.0000x reference)
"""DecisionGate (moe_routing) Bass kernel for 8 TRN2 NeuronCores.

Problem (hardcoded):
    x         [4096, 64]  f32
    act       [4096, 512] f32
    batch_inds[4096]      int64 (unused by the dense formulation)
Returns (g, mask, dispatched):
    g          [4096, 64]        f32   = 1 / (1 + x^4)
    mask       [4096, 64]        bool  = g >= 0.5
    dispatched [4096, 64, 512]   f32   = where(mask, g, 0)[:, :, None] * act[:, None, :]

Sharding: data parallel over batch B across 8 cores (512 rows/core).
Each core is fully independent (row-wise compute, no collectives).
The dominant cost is streaming the 64MB/core `dispatched` output to HBM,
so the kernel is structured as: act shard resident in SBUF, outer-product
tiles produced by VectorE (tensor_scalar, per-partition scalar) and
ScalarE (activation Copy with per-partition scale) in parallel, stored
with large 4MB HWDGE DMAs, triple buffered.
"""

import numpy as np

import concourse.bass as bass
import concourse.bacc as bacc
import concourse.mybir as mybir
from concourse.tile import TileContext
from concourse.bass_utils import run_bass_kernel_spmd

N_CORES = 8
B, P, D = 4096, 64, 512
BS = B // N_CORES          # 512 batch rows per core
NP = 128                   # SBUF partitions
NG = BS // NP              # 4 groups of 128 rows per core
PB = 16                    # p-block per output tile -> [128, PB*D] = 4MB tiles
THRES = 0.5

_cache = {}


def _build(reps=1, pb=PB, dbufs=3, mode="full", dma_engines=("sync",),
           io_engine="gpsimd", fine=False, mul_split=5, cbufs=None):
    f32 = mybir.dt.float32
    nc = bacc.Bacc(
        "TRN2",
        target_bir_lowering=False,
        debug=False,
        enable_asserts=False,
        num_devices=N_CORES,
    )
    x_d = nc.declare_dram_parameter("x", [BS, P], f32, isOutput=False)
    a_d = nc.declare_dram_parameter("act", [BS, D], f32, isOutput=False)
    g_d = nc.declare_dram_parameter("g", [BS, P], f32, isOutput=True)
    disp_d = nc.declare_dram_parameter("disp", [BS, P, D], f32, isOutput=True)

    # DRAM views: row (n*128 + p) -> partition p, free group n
    x_v = x_d[:].rearrange("(n p) m -> p n m", p=NP)        # [128, NG, 64]
    a_v = a_d[:].rearrange("(n p) d -> p n d", p=NP)        # [128, NG, 512]
    g_v = g_d[:].rearrange("(n p) m -> p n m", p=NP)

    with TileContext(nc) as tc:
        with (
            tc.tile_pool(name="small", bufs=1 if reps == 1 else 2) as small,
            tc.tile_pool(name="disp", bufs=dbufs) as dpool,
        ):
            def body(_i=None):
                x_t = small.tile([NP, NG * P], f32, tag="x")
                a_t = small.tile([NP, NG * D], f32, tag="a")
                t_t = small.tile([NP, NG * P], f32, tag="t")
                g_t = small.tile([NP, NG * P], f32, tag="g")
                w_t = small.tile([NP, NG * P], f32, tag="w")

                io_eng = getattr(nc, io_engine)
                io_eng.dma_start(
                    out=x_t[:].rearrange("p (n m) -> p n m", n=NG), in_=x_v
                )
                if fine:
                    for n in range(NG):
                        io_eng.dma_start(
                            out=a_t[:, n * D:(n + 1) * D], in_=a_v[:, n, :]
                        )
                    gslices = [
                        (slice(n * P, (n + 1) * P),) for n in range(NG)
                    ]
                else:
                    io_eng.dma_start(
                        out=a_t[:].rearrange("p (n d) -> p n d", n=NG), in_=a_v
                    )
                    gslices = [(slice(0, NG * P),)]

                # g = 1 / (1 + x^4); w = (g >= thres) * g
                for (sl,) in gslices:
                    nc.scalar.activation(
                        out=t_t[:, sl], in_=x_t[:, sl],
                        func=mybir.ActivationFunctionType.Square,
                    )
                    nc.scalar.activation(
                        out=t_t[:, sl], in_=t_t[:, sl],
                        func=mybir.ActivationFunctionType.Square,
                    )
                    nc.vector.tensor_scalar_add(
                        out=t_t[:, sl], in0=t_t[:, sl], scalar1=1.0
                    )
                    nc.vector.reciprocal(out=g_t[:, sl], in_=t_t[:, sl])
                    nc.vector.scalar_tensor_tensor(
                        out=w_t[:, sl],
                        in0=g_t[:, sl],
                        scalar=THRES,
                        in1=g_t[:, sl],
                        op0=mybir.AluOpType.is_ge,
                        op1=mybir.AluOpType.mult,
                    )
                io_eng.dma_start(
                    out=g_v, in_=g_t[:].rearrange("p (n m) -> p n m", n=NG)
                )

                # dispatched[n*128+q, p, :] = w[q, n*64+p] * act_row
                dma_i = 0
                for n in range(NG):
                    a_n = a_t[:, n * D:(n + 1) * D]
                    for j in range(P // pb):
                        d_t = dpool.tile([NP, pb * D], f32, tag="d")
                        if mode != "dma_only":
                            for k in range(pb):
                                p = j * pb + k
                                w_col = w_t[:, n * P + p:n * P + p + 1]
                                out_sl = d_t[:, k * D:(k + 1) * D]
                                # DVE fp32 tensor_scalar runs 2x/cycle
                                # @0.96GHz, ACT 1x @1.2GHz -> split ~10:6
                                if k % 8 < mul_split:
                                    nc.vector.tensor_scalar_mul(
                                        out=out_sl, in0=a_n, scalar1=w_col
                                    )
                                else:
                                    nc.scalar.mul(
                                        out=out_sl, in_=a_n, mul=w_col
                                    )
                        else:
                            # touch the tile once so it has a producer
                            nc.vector.tensor_scalar_mul(
                                out=d_t[:, 0:D], in0=a_n,
                                scalar1=w_t[:, 0:1],
                            )
                        if mode == "full_split2":
                            # both rings stream halves of the same tile
                            h = pb // 2
                            nc.sync.dma_start(
                                out=disp_d[
                                    n * NP:(n + 1) * NP, j * pb:j * pb + h, :
                                ],
                                in_=d_t[:, :h * D].rearrange(
                                    "q (a b) -> q a b", a=h
                                ),
                            )
                            nc.scalar.dma_start(
                                out=disp_d[
                                    n * NP:(n + 1) * NP,
                                    j * pb + h:(j + 1) * pb, :
                                ],
                                in_=d_t[:, h * D:].rearrange(
                                    "q (a b) -> q a b", a=h
                                ),
                            )
                        elif mode != "compute_only":
                            eng = getattr(nc, dma_engines[dma_i % len(dma_engines)])
                            dma_i += 1
                            eng.dma_start(
                                out=disp_d[
                                    n * NP:(n + 1) * NP, j * pb:(j + 1) * pb, :
                                ],
                                in_=d_t[:].rearrange("q (a b) -> q a b", a=pb),
                            )
                        else:
                            # tiny consumer so DCE keeps the compute
                            nc.sync.dma_start(
                                out=disp_d[n * NP:(n + 1) * NP, j * pb, :1],
                                in_=d_t[:, :1],
                            )

            if reps == 1:
                body()
            else:
                with tc.For_i(0, reps, 1) as i:
                    body(i)
    nc.compile()
    return nc


def kernel(x, act, batch_inds=None, _trace=False, _results_out=None, **_kw):
    x = np.ascontiguousarray(np.asarray(x, dtype=np.float32))
    act = np.ascontiguousarray(np.asarray(act, dtype=np.float32))
    assert x.shape == (B, P) and act.shape == (B, D), (x.shape, act.shape)

    if "nc" not in _cache:
        _cache["nc"] = _build()
    nc = _cache["nc"]

    in_maps = [
        {
            "x": np.ascontiguousarray(x[i * BS:(i + 1) * BS]),
            "act": np.ascontiguousarray(act[i * BS:(i + 1) * BS]),
        }
        for i in range(N_CORES)
    ]
    try:
        res = run_bass_kernel_spmd(nc, in_maps, list(range(N_CORES)), trace=_trace)
    except ModuleNotFoundError:
        # axon client without the NTFF profile hook — run untraced
        res = run_bass_kernel_spmd(nc, in_maps, list(range(N_CORES)), trace=False)
    if _results_out is not None:
        _results_out["bass_results"] = res

    g = np.concatenate([r["g"] for r in res.results], axis=0)
    dispatched = np.concatenate([r["disp"] for r in res.results], axis=0)
    mask = g >= np.float32(THRES)
    return g, mask, dispatched
